# revision 1
# baseline (speedup 1.0000x reference)
"""4-layer GraphSAGE (mean aggr) on 8 TRN2 NeuronCores.

Strategy (graph/data parallel, dst-owner node partitioning):
  - Nodes are partitioned across the 8 cores by dst ownership (12500 each,
    padded to 12544 = 98*128).  Each core aggregates the in-edges of its own
    nodes: per-edge dma_gather of src features from a replicated node-feature
    table in DRAM, then dma_scatter_add into per-core accumulators.
  - The gather index is int16, so the 100352-row table is addressed in 4
    ranges of 25088 rows; edges are grouped per (core, src-range) host-side.
  - dma_scatter_add's RMW is not atomic within one instruction, so each
    1024-edge chunk holds at most one edge per dst (host-side dealing).
    Across instructions the WAW chain serializes at ~8us/link, so chunks
    round-robin over NACC independent accumulators (tree-added on readback).
  - Layer 1 is "transform-first": table1 = x @ Wl1 (so every layer gathers
    128-wide rows), self term uses x^T directly.
  - Phase C per 512-node chunk: read back NACC partials, tree-add,
    normalize by 1/deg (broadcast mul), PE-transpose to feature-major,
    weight-stationary matmuls (agg @ Wl + h @ Wr + b, ReLU between layers),
    transpose back to node-major, AllGather shards into the next table.
    h^T is kept feature-major in DRAM and streamed per chunk.
"""

import numpy as np

# ---------------------------------------------------------------- constants
NCORES = 8
N = 100000
E = 1600000
F_IN = 16
H = 128
SHARD = 12500            # real nodes owned per core
BLK = 128
NBLK = 98                # 98*128 = 12544
SHARD_P = NBLK * BLK     # padded shard rows
TBL_ROWS = NCORES * SHARD_P   # 100352
NRANGE = 4
RANGE_ROWS = TBL_ROWS // NRANGE  # 25088 (< 2**15)
CHUNK = 1024             # edges per gather/scatter instruction (HW limit)
NCHUNK_R = 50            # chunks per (core, src-range)
CAP_R = NCHUNK_R * CHUNK  # 51200 edge capacity per (core, src-range)
NACC = 4                 # parallel scatter accumulators
JUNK_ROW = SHARD_P - 1   # scatter target for padding edges (a pad node)

_compiled = None


# ---------------------------------------------------------------- program
def _build_program(no_cc=False, gathers=True, scatters=True, phase_c=True,
                   repeat=1):
    import concourse.bacc as bacc
    import concourse.masks as masks
    import concourse.mybir as mybir
    import concourse.tile as tile

    fp32 = mybir.dt.float32
    i16 = mybir.dt.int16
    AF = mybir.ActivationFunctionType

    nc = bacc.Bacc(
        "TRN2",
        target_bir_lowering=False,
        debug=False,
        enable_asserts=False,
        num_devices=NCORES,
    )

    # -------- I/O declarations
    xt_d = nc.dram_tensor("xt", [F_IN, SHARD_P], fp32, kind="ExternalInput")
    # per chunk: 128 idx columns = [64 gather | 64 scatter], each wrapped
    # [16, 64] and replicated over the 8 16-partition groups
    idx_d = nc.dram_tensor(
        "idx", [128, NRANGE * NCHUNK_R * 128], i16, kind="ExternalInput"
    )
    invc_d = nc.dram_tensor("invc", [128, NBLK], fp32, kind="ExternalInput")
    w_d = {}
    for l in range(1, 5):
        din = F_IN if l == 1 else H
        w_d[f"wl{l}"] = nc.dram_tensor(f"wl{l}", [din, H], fp32, kind="ExternalInput")
        w_d[f"wr{l}"] = nc.dram_tensor(f"wr{l}", [din, H], fp32, kind="ExternalInput")
        w_d[f"b{l}"] = nc.dram_tensor(f"b{l}", [128, 1], fp32, kind="ExternalInput")

    out_d = nc.dram_tensor("out", [SHARD_P, H], fp32, kind="ExternalOutput")

    with tile.TileContext(nc) as tc:
        with (
            tc.tile_pool(name="dram", bufs=1, space="DRAM") as dpool,
            tc.tile_pool(name="const", bufs=1) as cpool,
            tc.tile_pool(name="gat", bufs=6) as gpool,
            tc.tile_pool(name="ix", bufs=8) as xpool,
            tc.tile_pool(name="agg", bufs=4) as apool,
            tc.tile_pool(name="work", bufs=3) as wpool,
            tc.tile_pool(name="psum_o", bufs=2, space="PSUM") as popool,
            tc.tile_pool(name="psum_t", bufs=2, space="PSUM") as ptpool,
        ):
            sh = [dpool.tile([SHARD_P, H], fp32, name=f"sh{l}") for l in range(4)]
            acc_d = [
                [dpool.tile([SHARD_P, H], fp32, name=f"acc{l}_{p}")
                 for p in range(NACC)]
                for l in range(1, 5)
            ]
            ntbl = repeat if not no_cc else 1
            tbls = [
                [dpool.tile([TBL_ROWS, H], fp32,
                            addr_space=("Local" if no_cc else "Shared"),
                            name=f"tbl{l}_r{r}")
                 for l in range(1, 5)]
                for r in range(ntbl)
            ]
            # feature-major h^T, streamed per chunk
            ht_dram = [
                dpool.tile([128, SHARD_P], fp32, name=f"ht{l}") for l in range(1, 4)
            ]

            # -------- constants to SBUF
            ident = cpool.tile([128, 128], fp32)
            masks.make_identity(nc, ident[:])
            xt_sb = cpool.tile([F_IN, SHARD_P], fp32)
            nc.sync.dma_start(xt_sb[:], xt_d.ap())
            invc_sb = cpool.tile([128, NBLK], fp32)
            nc.sync.dma_start(invc_sb[:], invc_d.ap())
            w_sb = {}
            for l in range(1, 5):
                din = F_IN if l == 1 else H
                for nm in (f"wl{l}", f"wr{l}"):
                    t = cpool.tile([din, H], fp32, name=f"{nm}_sb")
                    nc.sync.dma_start(t[:], w_d[nm].ap())
                    w_sb[nm] = t
                t = cpool.tile([128, 1], fp32, name=f"b{l}_sb")
                nc.sync.dma_start(t[:], w_d[f"b{l}"].ap())
                w_sb[f"b{l}"] = t

            # node-major view of DRAM row blocks: row n = b*128 + p
            def nm_view(dram_ap):
                return dram_ap.rearrange("(b p) f -> p b f", p=128)

            groups = [(i, min(4, NBLK - i)) for i in range(0, NBLK, 4)]

            def emit_nm(src_sb, cw, dst_view, b0, nb, tag):
                """src_sb [128f, cw] feature-major chunk -> node-major DRAM
                rows (blocks b0..b0+nb) via PE transposes."""
                pt = ptpool.tile([128, 4, 128], fp32, tag="pt")
                for j in range(nb):
                    nc.tensor.transpose(
                        pt[:, j, :], src_sb[:, j * 128 : (j + 1) * 128], ident[:]
                    )
                stage = wpool.tile([128, 4, 128], fp32, tag=f"nm_{tag}")
                nc.vector.tensor_copy(stage[:, :nb, :], pt[:, :nb, :])
                nc.sync.dma_start(dst_view[:, b0 : b0 + nb, :], stage[:, :nb, :])

            def allgather(src, dst):
                if no_cc:
                    nc.sync.dma_start(dst[:SHARD_P, :], src[:, :])
                    return
                nc.gpsimd.collective_compute(
                    "AllGather",
                    mybir.AluOpType.bypass,
                    replica_groups=[list(range(NCORES))],
                    ins=[src.opt()],
                    outs=[dst.opt()],
                )

            zt = cpool.tile([128, 14, 128], fp32, name="zt")
            nc.gpsimd.memset(zt[:], 0.0)

            for _rep in range(repeat):
                tbl = tbls[_rep % ntbl]
                # zero the scatter accumulators (no deps: overlaps freely)
                for accs in acc_d:
                    for a in accs:
                        av = nm_view(a)
                        for z in range(0, NBLK, 14):
                            nc.sync.dma_start(av[:, z : z + 14, :], zt[:])

                # ---- layer 1 transform: table1 = x @ Wl1
                sh0v = nm_view(sh[0])
                for b0, nb in groups:
                    cw = nb * 128
                    sl = slice(b0 * 128, b0 * 128 + cw)
                    ps = popool.tile([128, 512], fp32, tag="ps")
                    nc.tensor.matmul(
                        ps[:, :cw], w_sb["wl1"][:], xt_sb[:, sl],
                        start=True, stop=True,
                    )
                    tmp = wpool.tile([128, 512], fp32, tag="x1tmp")
                    nc.scalar.copy(tmp[:, :cw], ps[:, :cw])
                    emit_nm(tmp, cw, sh0v, b0, nb, "x1")
                allgather(sh[0], tbl[0])

                # ---- layers
                for l in range(1, 5):
                    table = tbl[l - 1]
                    accs = acc_d[l - 1]

                    # phase B: gather + scatter-add over all edges.  Each
                    # chunk has at most one edge per dst; chunks round-robin
                    # over the NACC accumulators to parallelize WAW chains.
                    for g in range(NRANGE):
                        tslice = table[g * RANGE_ROWS : (g + 1) * RANGE_ROWS, :]
                        for ci in range(NCHUNK_R):
                            q = g * NCHUNK_R + ci
                            ix = xpool.tile([128, 128], i16, tag="ix")
                            nc.sync.dma_start(
                                ix[:], idx_d.ap()[:, q * 128 : (q + 1) * 128]
                            )
                            gt = gpool.tile([128, CHUNK // 128, H], fp32, tag="gt")
                            if gathers:
                                nc.gpsimd.dma_gather(
                                    gt[:], tslice, ix[:, :64],
                                    num_idxs=CHUNK, num_idxs_reg=CHUNK,
                                    elem_size=H,
                                )
                            else:
                                nc.vector.memset(gt[:], 0.0)
                            if scatters:
                                nc.gpsimd.dma_scatter_add(
                                    accs[q % NACC][:, :],
                                    gt[:], ix[:, 64:128],
                                    num_idxs=CHUNK, num_idxs_reg=CHUNK,
                                    elem_size=H,
                                )

                    # phase C
                    if not phase_c:
                        continue
                    accvs = [nm_view(a) for a in accs]
                    dst_view = nm_view(sh[l] if l < 4 else out_d.ap())
                    SG = 14
                    for z0 in range(0, NBLK, SG):
                        parts = []
                        for p in range(NACC):
                            t = apool.tile([128, SG, 128], fp32, tag="agg_in")
                            nc.sync.dma_start(t[:], accvs[p][:, z0 : z0 + SG, :])
                            parts.append(t)
                        while len(parts) > 1:
                            nxt = []
                            for i in range(0, len(parts), 2):
                                a, b = parts[i], parts[i + 1]
                                nc.vector.tensor_add(a[:], a[:], b[:])
                                nxt.append(a)
                            parts = nxt
                        agg = parts[0]
                        scale = invc_sb[:, z0 : z0 + SG].to_broadcast(
                            [128, SG, 128]
                        )
                        nc.vector.tensor_mul(agg[:], agg[:], scale)
                        if l > 1:
                            hc = wpool.tile([128, SG, 128], fp32, tag="hc", bufs=2)
                            nc.sync.dma_start(
                                hc.rearrange("p a b -> p (a b)"),
                                ht_dram[l - 2][:, z0 * 128 : (z0 + SG) * 128],
                            )
                        ev = wpool.tile([128, SG, 128], fp32, tag="ev", bufs=2)
                        aggT = wpool.tile([128, SG, 128], fp32, tag="aggT", bufs=2)
                        stage = wpool.tile([128, SG, 128], fp32, tag="stage", bufs=2)
                        for s0 in range(0, SG, 4):
                            nb = min(4, SG - s0)
                            cw = nb * 128
                            pt = ptpool.tile([128, 4, 128], fp32, tag="pt")
                            for j in range(nb):
                                nc.tensor.transpose(
                                    pt[:, j, :], agg[:, s0 + j, :], ident[:]
                                )
                            nc.vector.tensor_copy(
                                aggT[:, s0 : s0 + nb, :], pt[:, :nb, :]
                            )
                            aggTf = aggT.rearrange("p a b -> p (a b)")
                            ps = popool.tile([128, 512], fp32, tag="ps")
                            fsl = slice(s0 * 128, s0 * 128 + cw)
                            if l == 1:
                                nc.tensor.matmul(
                                    ps[:, :cw], w_sb["wr1"][:],
                                    xt_sb[:, (z0 + s0) * 128 :
                                          (z0 + s0) * 128 + cw],
                                    start=True, stop=False,
                                )
                                nc.tensor.matmul(
                                    ps[:, :cw], ident[:], aggTf[:, fsl],
                                    start=False, stop=True,
                                )
                            else:
                                hcf = hc.rearrange("p a b -> p (a b)")
                                nc.tensor.matmul(
                                    ps[:, :cw], w_sb[f"wl{l}"][:], aggTf[:, fsl],
                                    start=True, stop=False,
                                )
                                nc.tensor.matmul(
                                    ps[:, :cw], w_sb[f"wr{l}"][:], hcf[:, fsl],
                                    start=False, stop=True,
                                )
                            func = AF.Relu if l < 4 else AF.Identity
                            nc.scalar.activation(
                                ev.rearrange("p a b -> p (a b)")[:, fsl],
                                ps[:, :cw], func, bias=w_sb[f"b{l}"][:],
                            )
                            pt2 = ptpool.tile([128, 4, 128], fp32, tag="pt")
                            for j in range(nb):
                                nc.tensor.transpose(
                                    pt2[:, j, :], ev[:, s0 + j, :], ident[:]
                                )
                            nc.vector.tensor_copy(
                                stage[:, s0 : s0 + nb, :], pt2[:, :nb, :]
                            )
                        if l < 4:
                            nc.sync.dma_start(
                                ht_dram[l - 1][:, z0 * 128 : (z0 + SG) * 128],
                                ev.rearrange("p a b -> p (a b)"),
                            )
                        nc.sync.dma_start(dst_view[:, z0 : z0 + SG, :], stage[:])

                    if l < 4:
                        allgather(sh[l], tbl[l])

    nc.compile()
    return nc


def _get_program():
    global _compiled
    if _compiled is None:
        _compiled = _build_program()
    return _compiled


# ---------------------------------------------------------------- host side
def _wrap_idx(a):
    """[L] int16 -> [128, L/16] layout: idx j at [j%16, j//16], replicated
    across the 8 groups of 16 partitions."""
    a2 = a.reshape(-1, 16).T.copy()
    return np.tile(a2, (8, 1))


def make_in_maps(x, edge_index, weights):
    src = np.asarray(edge_index[0], dtype=np.int64)
    dst = np.asarray(edge_index[1], dtype=np.int64)
    x = np.asarray(x, dtype=np.float32)

    cnt = np.bincount(dst, minlength=N).astype(np.float32)
    inv_full = (1.0 / np.maximum(cnt, 1.0)).astype(np.float32)

    core = dst // SHARD
    dst_loc = (dst - core * SHARD).astype(np.int64)
    src_row = (src // SHARD) * SHARD_P + (src % SHARD)
    rng = src_row // RANGE_ROWS
    src_loc = (src_row - rng * RANGE_ROWS).astype(np.int64)

    in_maps = []
    for c in range(NCORES):
        m = core == c
        gi = np.zeros(NRANGE * CAP_R, np.int16)
        si = np.full(NRANGE * CAP_R, JUNK_ROW, np.int16)
        for g in range(NRANGE):
            sel = m & (rng == g)
            k = int(sel.sum())
            assert k <= CAP_R, f"core {c} range {g}: {k} > {CAP_R}"
            s_g = src_loc[sel]
            d_g = dst_loc[sel]
            # group edges by dst, then deal to chunks round-robin: sorted
            # position i -> chunk i % NCHUNK_R.  Same-dst edges (consecutive
            # after the sort, degree <= NCHUNK_R) land in distinct chunks and
            # chunk loads are balanced to +-1.
            order = np.argsort(d_g, kind="stable")
            s_g, d_g = s_g[order], d_g[order]
            deg_max = np.bincount(d_g).max() if k else 0
            assert deg_max <= NCHUNK_R, f"deg {deg_max} > {NCHUNK_R}"
            chunk = np.arange(k) % NCHUNK_R
            # within each chunk, sort by src for gather locality
            order2 = np.lexsort((s_g, chunk))
            s_g, d_g, chunk = s_g[order2], d_g[order2], chunk[order2]
            loads = np.bincount(chunk, minlength=NCHUNK_R)
            starts = np.concatenate([[0], np.cumsum(loads)[:-1]])
            within = np.arange(k) - starts[chunk]
            slot = g * CAP_R + chunk * CHUNK + within
            gi[slot] = s_g.astype(np.int16)
            si[slot] = d_g.astype(np.int16)

        xt = np.zeros((F_IN, SHARD_P), np.float32)
        xt[:, :SHARD] = x[c * SHARD : (c + 1) * SHARD].T

        invc = np.zeros(SHARD_P, np.float32)
        invc[:SHARD] = inv_full[c * SHARD : (c + 1) * SHARD]
        invc = invc.reshape(NBLK, 128).T.copy()

        # merged per-chunk idx layout: chunk q -> cols [q*128, (q+1)*128),
        # first 64 = gather idx, last 64 = scatter idx, wrapped [16, 64] and
        # replicated across the 8 16-partition groups
        Q = NRANGE * NCHUNK_R
        G = gi.reshape(Q, 64, 16).transpose(0, 2, 1)
        S = si.reshape(Q, 64, 16).transpose(0, 2, 1)
        blk = np.concatenate([G, S], axis=2)
        blk = np.tile(blk, (1, 8, 1))
        idx_all = np.ascontiguousarray(
            blk.transpose(1, 0, 2).reshape(128, Q * 128)
        )
        im = {
            "xt": xt,
            "idx": idx_all,
            "invc": invc,
        }
        for l in range(1, 5):
            im[f"wl{l}"] = np.asarray(weights[f"Wl{l}"], np.float32)
            im[f"wr{l}"] = np.asarray(weights[f"Wr{l}"], np.float32)
            im[f"b{l}"] = np.asarray(weights[f"b{l}"], np.float32).reshape(128, 1)
        in_maps.append(im)
    return in_maps


def bench_exec(nc, in_maps, iters=5):
    """Mirror of bass2jax.run_bass_via_pjrt's multi-core path, but jits once,
    keeps inputs on device, and times repeated executions."""
    import time

    import jax
    import numpy as np_
    from jax.sharding import Mesh, PartitionSpec
    from jax.experimental.shard_map import shard_map

    from concourse import bass2jax, mybir

    bass2jax.install_neuronx_cc_hook()
    partition_name = (
        nc.partition_id_tensor.name if nc.partition_id_tensor else None
    )
    in_names, out_names, out_avals = [], [], []
    for alloc in nc.m.functions[0].allocations:
        if not isinstance(alloc, mybir.MemoryLocationSet):
            continue
        name = alloc.memorylocations[0].name
        if alloc.kind == "ExternalInput":
            if name != partition_name:
                in_names.append(name)
        elif alloc.kind == "ExternalOutput":
            out_names.append(name)
            out_avals.append(
                jax.core.ShapedArray(
                    tuple(alloc.tensor_shape), mybir.dt.np(alloc.dtype)
                )
            )
    n_params = len(in_names)
    all_in_names = list(in_names)
    if partition_name is not None:
        all_in_names.append(partition_name)

    def _body(*args):
        operands = list(args)
        if partition_name is not None:
            operands.append(bass2jax.partition_id_tensor())
        return tuple(
            bass2jax._bass_exec_p.bind(
                *operands,
                out_avals=tuple(out_avals),
                in_names=tuple(all_in_names),
                out_names=tuple(out_names),
                lowering_input_output_aliases=(),
                sim_require_finite=True,
                sim_require_nnan=True,
                nc=nc,
            )
        )

    n_cores = len(in_maps)
    devices = jax.devices()[:n_cores]
    mesh = Mesh(np_.asarray(devices), ("core",))
    fn = jax.jit(
        shard_map(
            _body,
            mesh=mesh,
            in_specs=(PartitionSpec("core"),) * n_params,
            out_specs=(PartitionSpec("core"),) * len(out_names),
            check_rep=False,
        ),
        keep_unused=True,
    )
    concat_in = [
        np_.concatenate([np_.asarray(in_maps[c][nm]) for c in range(n_cores)], axis=0)
        for nm in in_names
    ]
    dev_in = [jax.device_put(a) for a in concat_in]
    outs = fn(*dev_in)
    jax.block_until_ready(outs)
    times = []
    for _ in range(iters):
        t0 = time.perf_counter()
        outs = fn(*dev_in)
        jax.block_until_ready(outs)
        times.append(time.perf_counter() - t0)
    # async batch: dispatch many, block once — amortizes RPC overhead.
    # NOTE: unsafe with collectives (concurrent instances desync the mesh).
    if not nc.has_collectives:
        for nbatch in (8, 32):
            t0 = time.perf_counter()
            outss = [fn(*dev_in) for _ in range(nbatch)]
            jax.block_until_ready(outss)
            dt = time.perf_counter() - t0
            times.append(dt / nbatch)
    results = [
        {nm: np_.asarray(outs[i]).reshape(n_cores, *out_avals[i].shape)[c]
         for i, nm in enumerate(out_names)}
        for c in range(n_cores)
    ]
    return results, times


def kernel(x, edge_index, Wl1, Wr1, b1, Wl2, Wr2, b2, Wl3, Wr3, b3,
           Wl4, Wr4, b4, _trace=False, _trace_kwargs=None):
    from concourse.bass_utils import run_bass_kernel_spmd

    weights = {
        "Wl1": Wl1, "Wr1": Wr1, "b1": b1,
        "Wl2": Wl2, "Wr2": Wr2, "b2": b2,
        "Wl3": Wl3, "Wr3": Wr3, "b3": b3,
        "Wl4": Wl4, "Wr4": Wr4, "b4": b4,
    }
    nc = _get_program()
    in_maps = make_in_maps(x, edge_index, weights)
    res = run_bass_kernel_spmd(
        nc,
        in_maps,
        core_ids=list(range(NCORES)),
        trace=_trace,
        **(_trace_kwargs or {}),
    )
    shards = [res.results[c]["out"][:SHARD] for c in range(NCORES)]
    out = np.concatenate(shards, axis=0).astype(np.float32)
    if _trace:
        return out, res
    return out



# revision 13
# speedup vs baseline: 210.8151x; 210.8151x over previous
"""4-layer GraphSAGE (mean aggr) on 8 TRN2 NeuronCores — gather + PE segsum.

Strategy (dst-owner node partitioning, no scatter):
  - Nodes partitioned across 8 cores (12500 each, padded to 12544 = 98*128).
    Each core owns 98 dst blocks of 128 nodes.
  - Per layer, a replicated DRAM table holds TRANSFORMED features
    (table_l = h_{l-1} @ Wl_l, bf16), built shard-wise and AllGathered.
    Gathering transformed 128-wide rows makes the segment-sum directly
    produce mean@Wl.
  - Edges are grouped host-side by (dst block, src range) cells — 4 ranges
    of 25088 table rows keep gather indices int16. Each cell is padded to
    CAPG groups of 128 edges.
  - Aggregation is a tensor-engine segment-sum: for each 128-edge group, a
    one-hot matrix S[e, d] = (dst_rel[e] == d) * invdeg[e] is built on the
    vector engine (one is_equal against a replicated iota + one multiply),
    then matmul-accumulated into PSUM. No dma_scatter_add, no WAW chains.
  - Layers 1-3 accumulate feature-major psum [f, dst]: segsum (G as lhsT)
    + self term (Wr as lhsT, hT_prev as rhs), evicted with fused
    bias+ReLU (Act engine, per-partition bias) straight into an SBUF-resident
    feature-major hT. The next table (h @ Wl_{l+1}) is one more matmul per
    block (hT as lhsT) producing node-major rows — zero PE transposes.
  - Layer 4 accumulates node-major [dst, f] (S as lhsT); bias is a rank-1
    matmul (ones x b4^T); evicted fp32 to the output.
"""

import numpy as np

# ---------------------------------------------------------------- constants
NCORES = 8
N = 100000
E = 1600000
F_IN = 16
H = 128
SHARD = 12500            # real nodes owned per core
BLK = 128
NBLK = 98                # 98*128 = 12544
SHARD_P = NBLK * BLK     # padded shard rows
TBL_ROWS = NCORES * SHARD_P   # 100352
NRANGE = 4
RANGE_ROWS = TBL_ROWS // NRANGE  # 25088 (< 2**15)
NG_CHUNK = 7             # gather groups (of 128 edges) per gather instruction
                         # (896 idxs <= 1024 HW limit per instruction)

_compiled = {}


# ---------------------------------------------------------------- program
def _build_program(capg, repeat=1):
    import concourse.bacc as bacc
    import concourse.mybir as mybir
    import concourse.tile as tile

    fp32 = mybir.dt.float32
    bf16 = mybir.dt.bfloat16
    i16 = mybir.dt.int16
    AF = mybir.ActivationFunctionType
    EQ = mybir.AluOpType.is_equal

    GPB = NRANGE * capg          # groups per block (across ranges)
    NGRP = NBLK * GPB            # total groups (= dr/iv columns)
    SGRP_R = NBLK * capg         # groups per range stream
    assert SGRP_R % NG_CHUNK == 0
    NCHUNKS_R = SGRP_R // NG_CHUNK
    IDXC_R = SGRP_R * 8          # idx cols (16-wide wrap) per range

    nc = bacc.Bacc(
        "TRN2",
        target_bir_lowering=False,
        debug=False,
        enable_asserts=False,
        num_devices=NCORES,
    )

    # -------- I/O declarations
    xt_d = nc.dram_tensor("xt", [F_IN, SHARD_P], bf16, kind="ExternalInput")
    idx_d = nc.dram_tensor("idx", [128, NRANGE * IDXC_R], i16, kind="ExternalInput")
    dr_d = nc.dram_tensor("dr", [128, NGRP], bf16, kind="ExternalInput")
    iv_d = nc.dram_tensor("iv", [128, NGRP], bf16, kind="ExternalInput")
    iota_d = nc.dram_tensor("iota", [128, 128], bf16, kind="ExternalInput")
    w_d = {}
    for l in range(1, 5):
        din = F_IN if l == 1 else H
        w_d[f"wl{l}"] = nc.dram_tensor(f"wl{l}", [din, H], bf16, kind="ExternalInput")
        w_d[f"wr{l}"] = nc.dram_tensor(f"wr{l}", [din, H], bf16, kind="ExternalInput")
    for l in range(1, 4):
        w_d[f"bc{l}"] = nc.dram_tensor(f"bc{l}", [128, 1], fp32, kind="ExternalInput")
    w_d["b4b"] = nc.dram_tensor("b4b", [128, 128], fp32, kind="ExternalInput")

    out_d = nc.dram_tensor("out", [SHARD_P, H], fp32, kind="ExternalOutput")

    with tile.TileContext(nc) as tc:
        with (
            tc.tile_pool(name="dram", bufs=1, space="DRAM") as dpool,
            tc.tile_pool(name="const", bufs=1) as cpool,
            tc.tile_pool(name="g0", bufs=3) as gp0,
            tc.tile_pool(name="g1", bufs=3) as gp1,
            tc.tile_pool(name="g2", bufs=3) as gp2,
            tc.tile_pool(name="g3", bufs=3) as gp3,
            tc.tile_pool(name="sp", bufs=3) as spool,
            tc.tile_pool(name="work", bufs=3) as wpool,
            tc.tile_pool(name="psum_a", bufs=4, space="PSUM") as popool,
            tc.tile_pool(name="psum_t", bufs=2, space="PSUM") as ptpool,
        ):
            gpools = [gp0, gp1, gp2, gp3]
            # Shared tiles may only be written by one instruction each, so
            # tables (AllGather outputs) are allocated per repeat.
            tbls_r = [
                [dpool.tile([TBL_ROWS, H], bf16, addr_space="Shared",
                            name=f"tbl{l}_r{rep}")
                 for l in range(4)]
                for rep in range(repeat)
            ]
            shs_r = [
                [dpool.tile([SHARD_P, H], bf16, name=f"sh{l}_r{rep}")
                 for l in range(4)]
                for rep in range(repeat)
            ]

            # -------- constants to SBUF
            idx_sb = cpool.tile([128, NRANGE * IDXC_R], i16, name="idx_sb")
            nc.sync.dma_start(idx_sb[:], idx_d.ap())
            dr_sb = cpool.tile([128, NGRP], bf16, name="dr_sb")
            nc.sync.dma_start(dr_sb[:], dr_d.ap())
            iv_sb = cpool.tile([128, NGRP], bf16, name="iv_sb")
            nc.sync.dma_start(iv_sb[:], iv_d.ap())
            iota1 = cpool.tile([128, 128], bf16, name="iota1")
            nc.sync.dma_start(iota1[:], iota_d.ap())
            iota20 = cpool.tile([128, GPB, 128], bf16, name="iota20")
            for j in range(GPB):
                nc.vector.tensor_copy(iota20[:, j, :], iota1[:])
            xt_sb = cpool.tile([F_IN, SHARD_P], bf16, name="xt_sb")
            nc.sync.dma_start(xt_sb[:], xt_d.ap())
            w_sb = {}
            for l in range(1, 5):
                din = F_IN if l == 1 else H
                for nm in (f"wl{l}", f"wr{l}"):
                    t = cpool.tile([din, H], bf16, name=f"{nm}_sb")
                    nc.sync.dma_start(t[:], w_d[nm].ap())
                    w_sb[nm] = t
            for l in range(1, 4):
                t = cpool.tile([128, 1], fp32, name=f"bc{l}_sb")
                nc.sync.dma_start(t[:], w_d[f"bc{l}"].ap())
                w_sb[f"bc{l}"] = t
            b4b_sb = cpool.tile([128, 128], fp32, name="b4b_sb")
            nc.sync.dma_start(b4b_sb[:], w_d["b4b"].ap())

            # feature-major hidden state, SBUF resident, ping-pong
            hA = cpool.tile([128, SHARD_P], bf16, name="hA")
            hB = cpool.tile([128, SHARD_P], bf16, name="hB")

            def nm_view(t):
                return t.rearrange("(b p) f -> p b f", p=128)

            out_v = nm_view(out_d.ap())

            def allgather(src, dst):
                nc.gpsimd.collective_compute(
                    "AllGather",
                    mybir.AluOpType.bypass,
                    replica_groups=[list(range(NCORES))],
                    ins=[src.opt()],
                    outs=[dst.opt()],
                )

            for _rep in range(repeat):
                tbls = tbls_r[_rep]
                shs = shs_r[_rep]
                sh_v = [nm_view(s) for s in shs]
                # ---- phase 0: table1 = x @ Wl1 (node-major shard, AllGather)
                for b in range(NBLK):
                    cols = slice(b * 128, (b + 1) * 128)
                    ps0 = popool.tile([128, 128], fp32, tag="ps")
                    nc.tensor.matmul(
                        ps0[:], xt_sb[:, cols], w_sb["wl1"][:],
                        start=True, stop=True,
                    )
                    st = wpool.tile([128, 128], bf16, tag="st")
                    nc.scalar.copy(st[:], ps0[:])
                    nc.sync.dma_start(sh_v[0][:, b, :], st[:])
                allgather(shs[0], tbls[0])

                # ---- layers
                for l in range(1, 5):
                    tbl_in = tbls[l - 1]
                    hT_prev = [xt_sb, hA, hB, hA][l - 1]
                    hT_next = [hA, hB, hA, None][l - 1]
                    gt = [[None] * NCHUNKS_R for _ in range(NRANGE)]
                    pending = []

                    def emit_block(b, l=l, gt=gt, hT_prev=hT_prev,
                                   hT_next=hT_next, pending=pending):
                        cols = slice(b * 128, (b + 1) * 128)
                        gsl = slice(b * GPB, (b + 1) * GPB)
                        S = spool.tile([128, GPB, 128], bf16, tag="s")
                        nc.vector.tensor_tensor(
                            S[:], iota20[:],
                            dr_sb[:, gsl].to_broadcast([128, GPB, 128]), EQ,
                        )
                        nc.vector.tensor_mul(
                            S[:], S[:],
                            iv_sb[:, gsl].to_broadcast([128, GPB, 128]),
                        )
                        ps = popool.tile([128, 128], fp32, tag="ps")
                        first = True
                        for r in range(NRANGE):
                            for k in range(capg):
                                g = b * capg + k
                                c, pos = divmod(g, NG_CHUNK)
                                G = gt[r][c]
                                if l < 4:
                                    nc.tensor.matmul(
                                        ps[:], G[:, pos, :],
                                        S[:, r * capg + k, :],
                                        start=first, stop=False,
                                    )
                                else:
                                    nc.tensor.matmul(
                                        ps[:], S[:, r * capg + k, :],
                                        G[:, pos, :],
                                        start=first, stop=False,
                                    )
                                first = False
                        if l < 4:
                            nc.tensor.matmul(
                                ps[:], w_sb[f"wr{l}"][:], hT_prev[:, cols],
                                start=False, stop=True,
                            )
                            nc.scalar.activation(
                                hT_next[:, cols], ps[:], AF.Relu,
                                bias=w_sb[f"bc{l}"][:],
                            )

                            def mk(b=b, l=l, hT_next=hT_next):
                                cols = slice(b * 128, (b + 1) * 128)
                                ps2 = ptpool.tile([128, 128], fp32, tag="ps2")
                                nc.tensor.matmul(
                                    ps2[:], hT_next[:, cols],
                                    w_sb[f"wl{l + 1}"][:],
                                    start=True, stop=True,
                                )
                                st = wpool.tile([128, 128], bf16, tag="st")
                                nc.scalar.copy(st[:], ps2[:])
                                nc.sync.dma_start(sh_v[l][:, b, :], st[:])

                            pending.append(mk)
                            if len(pending) >= 2:
                                pending.pop(0)()
                        else:
                            nc.tensor.matmul(
                                ps[:], hT_prev[:, cols], w_sb["wr4"][:],
                                start=False, stop=True,
                            )
                            st = wpool.tile([128, 128], fp32, tag="ost")
                            nc.vector.tensor_add(st[:], ps[:], b4b_sb[:])
                            nc.sync.dma_start(out_v[:, b, :], st[:])

                    nextb = 0
                    for c in range(NCHUNKS_R):
                        for r in range(NRANGE):
                            G = gpools[r].tile(
                                [128, NG_CHUNK, 128], bf16, tag=f"g{r}"
                            )
                            base = r * IDXC_R + c * NG_CHUNK * 8
                            nc.gpsimd.dma_gather(
                                G[:],
                                tbl_in[r * RANGE_ROWS : (r + 1) * RANGE_ROWS, :],
                                idx_sb[:, base : base + NG_CHUNK * 8],
                                num_idxs=NG_CHUNK * 128,
                                num_idxs_reg=NG_CHUNK * 128,
                                elem_size=H,
                            )
                            gt[r][c] = G
                        while (nextb < NBLK
                               and ((nextb + 1) * capg - 1) // NG_CHUNK <= c):
                            emit_block(nextb)
                            nextb += 1
                    assert nextb == NBLK
                    while pending:
                        pending.pop(0)()
                    if l < 4:
                        allgather(shs[l], tbls[l])

    nc.compile()
    return nc


def _get_program(capg, repeat=1):
    key = (capg, repeat)
    if key not in _compiled:
        _compiled[key] = _build_program(capg, repeat=repeat)
    return _compiled[key]


# ---------------------------------------------------------------- host side
def make_in_maps(x, edge_index, weights):
    from ml_dtypes import bfloat16

    src = np.asarray(edge_index[0]).astype(np.int64)
    dst = np.asarray(edge_index[1]).astype(np.int64)
    x = np.asarray(x, dtype=np.float32)

    cnt = np.bincount(dst, minlength=N).astype(np.float32)
    inv_full = (1.0 / np.maximum(cnt, 1.0)).astype(np.float32)

    core = dst // SHARD
    srow = (src // SHARD) * SHARD_P + (src % SHARD)
    rng = srow // RANGE_ROWS
    loc = (srow % RANGE_ROWS).astype(np.int64)

    # pass 1: global CAPG (same static structure on every core)
    capg = 0
    percore = []
    for c in range(NCORES):
        m = core == c
        dloc = dst[m] - c * SHARD
        cell = (dloc >> 7) * NRANGE + rng[m]
        cnts = np.bincount(cell, minlength=NBLK * NRANGE)
        capg = max(capg, int(-(-cnts.max() // 128)))
        percore.append((m, dloc, cell, cnts))
    assert capg <= 10, f"unexpectedly unbalanced graph: capg={capg}"

    GPB = NRANGE * capg
    NGRP = NBLK * GPB
    SGRP_R = NBLK * capg

    iota = np.tile(np.arange(128, dtype=np.float32), (128, 1)).astype(bfloat16)

    in_maps = []
    for c in range(NCORES):
        m, dloc, cell, cnts = percore[c]
        loc_c = loc[m]
        iv_e = inv_full[dst[m]]
        rel = (dloc & 127).astype(np.float32)

        order = np.lexsort((loc_c, cell))
        cell_o = cell[order]
        loc_o = loc_c[order]
        rel_o = rel[order]
        iv_o = iv_e[order]

        starts = np.concatenate([[0], np.cumsum(cnts)[:-1]])
        within = np.arange(cell_o.size, dtype=np.int64) - starts[cell_o]
        bb = cell_o // NRANGE
        rr = cell_o % NRANGE

        gi = np.zeros((NRANGE, SGRP_R * 128), np.int16)
        gi[rr, bb * (capg * 128) + within] = loc_o.astype(np.int16)

        k = within >> 7
        p = within & 127
        col = bb * GPB + rr * capg + k
        dr = np.full((128, NGRP), -1.0, np.float32)
        ivr = np.zeros((128, NGRP), np.float32)
        dr[p, col] = rel_o
        ivr[p, col] = iv_o

        idxw = np.ascontiguousarray(np.tile(np.concatenate(
            [gi[r].reshape(-1, 16).T for r in range(NRANGE)], axis=1
        ), (8, 1)))

        xt = np.zeros((F_IN, SHARD_P), np.float32)
        xt[:, :SHARD] = x[c * SHARD : (c + 1) * SHARD].T

        im = {
            "xt": xt.astype(bfloat16),
            "idx": idxw,
            "dr": dr.astype(bfloat16),
            "iv": ivr.astype(bfloat16),
            "iota": iota,
        }
        for l in range(1, 5):
            im[f"wl{l}"] = np.asarray(weights[f"Wl{l}"], np.float32).astype(bfloat16)
            im[f"wr{l}"] = np.asarray(weights[f"Wr{l}"], np.float32).astype(bfloat16)
        for l in range(1, 4):
            im[f"bc{l}"] = np.asarray(
                weights[f"b{l}"], np.float32).reshape(128, 1)
        im["b4b"] = np.ascontiguousarray(np.tile(
            np.asarray(weights["b4"], np.float32).reshape(1, 128), (128, 1)))
        in_maps.append(im)
    return in_maps, capg


def bench_exec(nc, in_maps, iters=5):
    """Mirror of bass2jax.run_bass_via_pjrt's multi-core path, but jits once,
    keeps inputs on device, and times repeated executions."""
    import time

    import jax
    import numpy as np_
    from jax.sharding import Mesh, PartitionSpec
    from jax.experimental.shard_map import shard_map

    from concourse import bass2jax, mybir

    bass2jax.install_neuronx_cc_hook()
    partition_name = (
        nc.partition_id_tensor.name if nc.partition_id_tensor else None
    )
    in_names, out_names, out_avals = [], [], []
    for alloc in nc.m.functions[0].allocations:
        if not isinstance(alloc, mybir.MemoryLocationSet):
            continue
        name = alloc.memorylocations[0].name
        if alloc.kind == "ExternalInput":
            if name != partition_name:
                in_names.append(name)
        elif alloc.kind == "ExternalOutput":
            out_names.append(name)
            out_avals.append(
                jax.core.ShapedArray(
                    tuple(alloc.tensor_shape), mybir.dt.np(alloc.dtype)
                )
            )
    n_params = len(in_names)
    all_in_names = list(in_names)
    if partition_name is not None:
        all_in_names.append(partition_name)

    def _body(*args):
        operands = list(args)
        if partition_name is not None:
            operands.append(bass2jax.partition_id_tensor())
        return tuple(
            bass2jax._bass_exec_p.bind(
                *operands,
                out_avals=tuple(out_avals),
                in_names=tuple(all_in_names),
                out_names=tuple(out_names),
                lowering_input_output_aliases=(),
                sim_require_finite=True,
                sim_require_nnan=True,
                nc=nc,
            )
        )

    n_cores = len(in_maps)
    devices = jax.devices()[:n_cores]
    mesh = Mesh(np_.asarray(devices), ("core",))
    fn = jax.jit(
        shard_map(
            _body,
            mesh=mesh,
            in_specs=(PartitionSpec("core"),) * n_params,
            out_specs=(PartitionSpec("core"),) * len(out_names),
            check_rep=False,
        ),
        keep_unused=True,
    )
    concat_in = [
        np_.concatenate([np_.asarray(in_maps[c][nm]) for c in range(n_cores)], axis=0)
        for nm in in_names
    ]
    dev_in = [jax.device_put(a) for a in concat_in]
    outs = fn(*dev_in)
    jax.block_until_ready(outs)
    times = []
    for _ in range(iters):
        t0 = time.perf_counter()
        outs = fn(*dev_in)
        jax.block_until_ready(outs)
        times.append(time.perf_counter() - t0)
    results = [
        {nm: np_.asarray(outs[i]).reshape(n_cores, *out_avals[i].shape)[c]
         for i, nm in enumerate(out_names)}
        for c in range(n_cores)
    ]
    return results, times


def kernel(x, edge_index, Wl1, Wr1, b1, Wl2, Wr2, b2, Wl3, Wr3, b3,
           Wl4, Wr4, b4, _trace=False, _trace_kwargs=None):
    from concourse.bass_utils import run_bass_kernel_spmd

    weights = {
        "Wl1": Wl1, "Wr1": Wr1, "b1": b1,
        "Wl2": Wl2, "Wr2": Wr2, "b2": b2,
        "Wl3": Wl3, "Wr3": Wr3, "b3": b3,
        "Wl4": Wl4, "Wr4": Wr4, "b4": b4,
    }
    in_maps, capg = make_in_maps(x, edge_index, weights)
    nc = _get_program(capg)
    res = run_bass_kernel_spmd(
        nc,
        in_maps,
        core_ids=list(range(NCORES)),
        trace=_trace,
        **(_trace_kwargs or {}),
    )
    shards = [res.results[c]["out"][:SHARD] for c in range(NCORES)]
    out = np.concatenate(shards, axis=0).astype(np.float32)
    if _trace:
        return out, res
    return out
